# revision 9
# baseline (speedup 1.0000x reference)
"""BoundaryLoss (EDT-weighted BCE) on 8 Trainium2 NeuronCores — v3.1.

Layout: per core, partitions = 64 D-rows x 2 channels INTERLEAVED in the
partition LSB (partition p = 2*row + ch; ch0 = target, ch1 = 1-target);
free dims = (H=96, W=24 interior + 8 halo). 8 cores = 2 batches x 4
W-quarters.

The interleave makes every D-axis shift a uniform partition shift of 2d
for both channels, so the D pass is 8 SBUF->SBUF DMA copies (one per
distance x direction x H-half) + tensor_tensor mins seeded with tiny
DRAM sentinel strips (rows shifted in from out-of-volume read 100 >= 25
= the w=0 saturation zone). No PE shifts, no Act drains, no per-channel
slot splits, no sub-64-partition ops (illegal off 0/32/64/96 bases).

Engine facts this build enforces: tensor_tensor(min) and scans are
DVE-only (Pool rejects them); Pool runs tensor_scalar / copies / tt
add/mult; tensor_scalar is 4x on DVE (0.26 ns/elem bf16), tensor_tensor
2x (0.52). So both scans and all 16 D/H window mins run on DVE; Act and
Pool take the +d^2 bias copies, the BCE Ln, and finalize stages.

W pass: fwd scan over cols [0:28), bwd scan (reversed APs) over [4:32)
— each direction only needs its upstream 4-col halo; chained row
chunks; cross-row leaks land >= 5 = saturation.

BCE: host packs q = where(target, pred, 1-pred) (duplicated to both
partition parities) as fp16; device does scr = Ln(q + 3.8e-44) on Act
(the tiny bias reproduces torch's log clamp at -100 for q = 0);
bce = -scr, sign folded into the host reduction. The fp16 (not bf16)
ln/product chain keeps the systematic rounding bias ~1e-5.

Finalize per H-half: PE matmul with a banded pair-sum matrix combines
the interleaved channels (psum[2r] = g[2r] + g[2r+1] = a^2, exactly one
of the pair is 0); Act reads PSUM: da = Sqrt(0.25*a^2) = a/2; DVE:
w = clamp(2.5 - da, 0, 1) via 4x tensor_scalar, den/num accumulated
per-partition via tensor_scalar accum_out (accum reduces with op1, so
both accum ops end in add). Host sums EVEN partitions only (odd ones
carry parity-op garbage) in float64:
loss = mean_b(num_b / (den_b + 1e-5)).
"""

import numpy as np

B, D, H, W = 2, 64, 96, 96
NQ = 4
WI = W // NQ       # 24 interior columns per core
HALO = 4
WE = WI + 2 * HALO  # 32
N_CORES = B * NQ
HH = H // 2

_CACHE = {}


def _build():
    import concourse.bacc as bacc
    import concourse.mybir as mybir
    import concourse.tile as tile

    fp32 = mybir.dt.float32
    bf16 = mybir.dt.bfloat16
    fp16 = mybir.dt.float16
    AF = mybir.ActivationFunctionType
    ALU = mybir.AluOpType

    nc = bacc.Bacc("TRN2", target_bir_lowering=False, debug=False)
    t_d = nc.dram_tensor("t", [128, H, WE], bf16, kind="ExternalInput").ap()
    q_d = nc.dram_tensor("q", [128, H, WI], fp16, kind="ExternalInput").ap()
    s_d = nc.dram_tensor("s", [8, H, WI], bf16, kind="ExternalInput").ap()
    m_d = nc.dram_tensor("pm", [128, 128], bf16, kind="ExternalInput").ap()
    o_d = nc.dram_tensor("o", [128, 4], fp32, kind="ExternalOutput").ap()

    with tile.TileContext(nc) as tc:
        with (
            tc.tile_pool(name="mem", bufs=1) as pool,
            tc.tile_pool(name="ps", bufs=2, space="PSUM") as psp,
        ):
            t2 = pool.tile([128, H, WE], bf16)
            sf = pool.tile([128, H, WE], bf16)
            sb = pool.tile([128, H, WE], bf16)
            m1 = pool.tile([128, H, WI], bf16)
            sq = pool.tile([128, H, WI], bf16)
            thd = [pool.tile([128, H, WI], bf16, name=f"thd{d}") for d in (1, 2, 3, 4)]
            su = [pool.tile([128, H, WI], bf16, name=f"su{d}") for d in (1, 2, 3, 4)]
            sd = [pool.tile([128, H, WI], bf16, name=f"sd{d}") for d in (1, 2, 3, 4)]
            gd = pool.tile([128, H, WI], bf16)
            the = [pool.tile([128, H, WI], bf16, name=f"the{d}") for d in (1, 2, 3, 4)]
            gh = pool.tile([128, H, WI], bf16)
            qt = pool.tile([128, H, WI], fp16)
            scr = pool.tile([128, H, WI], fp16)
            da = pool.tile([128, H, WI], bf16)
            uv = pool.tile([128, H, WI], fp16)
            wv = pool.tile([128, H, WI], fp16)
            mb = pool.tile([128, H, WI], fp16)
            pm = pool.tile([128, 128], bf16)
            eps = pool.tile([128, 1], fp32)
            os_ = pool.tile([128, 4], fp32)

            def tmin(out_ap, a_ap, b_ap):
                nc.vector.tensor_tensor(out_ap, a_ap, b_ap, op=ALU.min)

            nc.vector.memset(eps[:], 3.7835058e-44)

            # ---- loads: upper rows first (bwd scan starts there), aux
            # tensors on the Act queue.
            nc.sync.dma_start(t2[:, 72:96], t_d[:, 72:96])
            nc.sync.dma_start(t2[:, 48:72], t_d[:, 48:72])
            nc.sync.dma_start(t2[:, 0:24], t_d[:, 0:24])
            nc.sync.dma_start(t2[:, 24:48], t_d[:, 24:48])
            nc.scalar.dma_start(qt[:], q_d)
            nc.scalar.dma_start(pm[:], m_d)
            for k, d in enumerate((1, 2, 3, 4)):
                nc.scalar.dma_start(su[k][128 - 2 * d:128], s_d[0:2 * d])
                nc.scalar.dma_start(sd[k][0:2 * d], s_d[0:2 * d])
            nc.scalar.activation(os_[:, 0:1], eps[:], AF.Ln)  # pin ln table

            # ---- W pass: fwd + bwd distance scans, both DVE, ordered to
            # chase the t2 chunk loads. state = t*state + t.
            def fscan(a, b, seed):
                src = t2[:, a:b].opt()
                nc.vector.tensor_tensor_scan(
                    sf[:, a:b].opt(), src, src, seed,
                    op0=ALU.mult, op1=ALU.add)

            def bscan(a, b, seed):
                src = t2[:, a:b].opt()[:, ::-1]
                nc.vector.tensor_tensor_scan(
                    sb[:, a:b].opt()[:, ::-1], src, src, seed,
                    op0=ALU.mult, op1=ALU.add)

            bscan(72, 96, 100.0)
            bscan(48, 72, sb[:, 72:73, 0:1].opt())
            fscan(0, 24, 100.0)
            fscan(24, 48, sf[:, 23:24, WE - 1:WE].opt())
            fscan(48, 96, sf[:, 47:48, WE - 1:WE].opt())
            # m1/sq upper half while the lower bwd chunks are still queued
            tmin(m1[:, HH:H], sf[:, HH:H, 4:28], sb[:, HH:H, 4:28])
            nc.vector.tensor_tensor(sq[:, HH:H], m1[:, HH:H], m1[:, HH:H],
                                    op=ALU.mult)
            nc.vector.tensor_scalar(thd[3][:, HH:H], sq[:, HH:H], 1.0, 16.0,
                                    op0=ALU.mult, op1=ALU.add)
            bscan(24, 48, sb[:, 48:49, 0:1].opt())
            bscan(0, 24, sb[:, 24:25, 0:1].opt())
            tmin(m1[:, 0:HH], sf[:, 0:HH, 4:28], sb[:, 0:HH, 4:28])
            nc.vector.tensor_tensor(sq[:, 0:HH], m1[:, 0:HH], m1[:, 0:HH],
                                    op=ALU.mult)
            nc.vector.tensor_scalar(thd[3][:, 0:HH], sq[:, 0:HH], 1.0, 16.0,
                                    op0=ALU.mult, op1=ALU.add)

            # ---- D pass: +d^2 biases (d=4 DVE halves above; d=3,2 Act;
            # d=1 DVE filler), per-half 2d-partition shift DMAs, 16
            # half-tile DVE mins ordered d=4..1, up/down, hi/lo.
            nc.scalar.activation(thd[2][:], sq[:], AF.Copy, bias=9.0)
            nc.scalar.activation(thd[1][:], sq[:], AF.Copy, bias=4.0)
            halves = ((HH, H), (0, HH))
            for d in (4, 3, 2):
                k = d - 1
                for (a, b) in halves:
                    nc.sync.dma_start(su[k][0:128 - 2 * d, a:b],
                                      thd[k][2 * d:128, a:b])
                    nc.sync.dma_start(sd[k][2 * d:128, a:b],
                                      thd[k][0:128 - 2 * d, a:b])
            for (a, b) in halves:
                tmin(gd[:, a:b], sq[:, a:b], su[3][:, a:b])
            for (a, b) in halves:
                tmin(gd[:, a:b], sd[3][:, a:b], gd[:, a:b])
            # d=1 bias on DVE here: fills the gap while Act finishes thd3
            nc.vector.tensor_scalar(thd[0][:], sq[:], 1.0, 1.0,
                                    op0=ALU.mult, op1=ALU.add)
            for (a, b) in halves:
                nc.sync.dma_start(su[0][0:126, a:b], thd[0][2:128, a:b])
                nc.sync.dma_start(sd[0][2:128, a:b], thd[0][0:126, a:b])
            for s_ in (su[2], sd[2], su[1], sd[1], su[0], sd[0]):
                for (a, b) in halves:
                    tmin(gd[:, a:b], s_[:, a:b], gd[:, a:b])

            # ---- BCE on Act (q loaded long ago); pin the sqrt table
            # right after so the finalize Sqrt doesn't stall on a load.
            nc.scalar.activation(scr[:], qt[:], AF.Ln, bias=eps[:])
            nc.scalar.activation(os_[:, 1:2], eps[:], AF.Sqrt)

            # ---- H pass: free-dim shifted mins, d = 1..4, edge-trimmed.
            # Biases: d=1 DVE (needed first), d=2,3 Act, d=4 Pool.
            nc.vector.tensor_scalar(the[0][:], gd[:], 1.0, 1.0,
                                    op0=ALU.mult, op1=ALU.add)
            nc.scalar.activation(the[1][:], gd[:], AF.Copy, bias=4.0)
            nc.scalar.activation(the[2][:], gd[:], AF.Copy, bias=9.0)
            nc.gpsimd.tensor_scalar(the[3][:], gd[:], 1.0, 16.0,
                                    op0=ALU.mult, op1=ALU.add)
            nc.vector.tensor_copy(gh[:, 0:1, :], gd[:, 0:1, :])
            tmin(gh[:, 1:96], gd[:, 1:96], the[0][:, 0:95])
            tmin(gh[:, 0:95], the[0][:, 1:96], gh[:, 0:95])
            tmin(gh[:, 2:96], the[1][:, 0:94], gh[:, 2:96])
            tmin(gh[:, 0:94], the[1][:, 2:96], gh[:, 0:94])
            tmin(gh[:, 3:96], the[2][:, 0:93], gh[:, 3:96])
            tmin(gh[:, 0:93], the[2][:, 3:96], gh[:, 0:93])
            # d=4 split per half, upper first, so finalize(h1) overlaps
            # the lower-half tail.
            tmin(gh[:, HH:96], the[3][:, HH - 4:92], gh[:, HH:96])
            tmin(gh[:, HH:92], the[3][:, HH + 4:96], gh[:, HH:92])
            tmin(gh[:, 4:HH], the[3][:, 0:HH - 4], gh[:, 4:HH])
            tmin(gh[:, 0:HH], the[3][:, 4:HH + 4], gh[:, 0:HH])

            # ---- finalize per H-half (upper first): PE pair-sum, Act
            # sqrt from PSUM, DVE ramp + accumulates.
            FH = HH * WI  # 1152
            for i, (a, b) in enumerate(((HH, H), (0, HH))):
                pt = psp.tile([128, 3 * 512], fp32, name=f"ps{i}", tag="ps")
                ghf = gh[:, a:b, :].opt()
                for c0 in (0, 512, 1024):
                    c1 = min(c0 + 512, FH)
                    nc.tensor.matmul(pt[:, c0:c1], pm[:], ghf[:, c0:c1],
                                     start=True, stop=True)
                nc.scalar.activation(da[:, a:b, :].opt(), pt[:, 0:FH],
                                     AF.Sqrt, scale=0.25)
                nc.vector.tensor_scalar(uv[:, a:b], da[:, a:b], -1.0, 2.5,
                                        op0=ALU.mult, op1=ALU.add)
                nc.vector.tensor_scalar(wv[:, a:b], uv[:, a:b], 1.0, 0.0,
                                        op0=ALU.min, op1=ALU.max)
                nc.vector.tensor_scalar(uv[:, a:b], wv[:, a:b], 1.0, 0.0,
                                        op0=ALU.mult, op1=ALU.add,
                                        accum_out=os_[:, 2 * i:2 * i + 1])
                nc.vector.tensor_tensor(mb[:, a:b], wv[:, a:b], scr[:, a:b],
                                        op=ALU.mult)
                nc.vector.tensor_scalar(mb[:, a:b], mb[:, a:b], 1.0, 0.0,
                                        op0=ALU.mult, op1=ALU.add,
                                        accum_out=os_[:, 2 * i + 1:2 * i + 2])
                nc.sync.dma_start(o_d[:, 2 * i:2 * i + 2],
                                  os_[:, 2 * i:2 * i + 2])
    nc.compile()
    return nc


def _get_nc():
    if "nc" not in _CACHE:
        _CACHE["nc"] = _build()
    return _CACHE["nc"]


def _slabs(pred, target):
    import ml_dtypes

    bf16 = ml_dtypes.bfloat16
    tf = np.asarray(target, dtype=np.float32)
    tp = np.pad(tf, ((0, 0), (0, 0), (0, 0), (HALO, HALO)), mode="edge")
    qf = np.where(tf > 0.5, np.asarray(pred, dtype=np.float32),
                  1.0 - np.asarray(pred, dtype=np.float32))
    sent = np.full((8, H, WI), 100.0, dtype=bf16)
    pmat = np.zeros((128, 128), dtype=np.float32)
    for p in range(0, 128, 2):
        pmat[p, p] = 1.0
        pmat[p + 1, p] = 1.0
    pmat = pmat.astype(bf16)
    in_maps = []
    for b in range(B):
        for qr in range(NQ):
            ts_ = tp[b, :, :, qr * WI: qr * WI + WE]  # [64, H, WE]
            t2 = np.empty((128, H, WE), dtype=np.float32)
            t2[0::2] = ts_
            t2[1::2] = 1.0 - ts_
            qq = qf[b, :, :, qr * WI:(qr + 1) * WI]   # [64, H, WI]
            q2 = np.empty((128, H, WI), dtype=np.float32)
            q2[0::2] = qq
            q2[1::2] = qq
            in_maps.append({"t": t2.astype(bf16), "q": q2.astype(np.float16),
                            "s": sent, "pm": pmat})
    return in_maps


def kernel(pred: np.ndarray, target: np.ndarray) -> np.ndarray:
    from concourse.bass_utils import run_bass_kernel_spmd

    nc = _get_nc()
    in_maps = _slabs(pred, target)
    res = run_bass_kernel_spmd(nc, in_maps, list(range(N_CORES)))

    loss = 0.0
    for b in range(B):
        num = 0.0
        den = 0.0
        for qr in range(NQ):
            o = res.results[b * NQ + qr]["o"].astype(np.float64)[0::2]
            den += o[:, 0].sum() + o[:, 2].sum()
            num -= o[:, 1].sum() + o[:, 3].sum()
        loss += num / (den + 1e-5)
    return np.float32(loss / B)


# revision 10
# speedup vs baseline: 1.0448x; 1.0448x over previous
"""BoundaryLoss (EDT-weighted BCE) on 8 Trainium2 NeuronCores — v3.1.

Layout: per core, partitions = 64 D-rows x 2 channels INTERLEAVED in the
partition LSB (partition p = 2*row + ch; ch0 = target, ch1 = 1-target);
free dims = (H=96, W=24 interior + 8 halo). 8 cores = 2 batches x 4
W-quarters.

The interleave makes every D-axis shift a uniform partition shift of 2d
for both channels, so the D pass is 8 SBUF->SBUF DMA copies (one per
distance x direction x H-half) + tensor_tensor mins seeded with tiny
DRAM sentinel strips (rows shifted in from out-of-volume read 100 >= 25
= the w=0 saturation zone). No PE shifts, no Act drains, no per-channel
slot splits, no sub-64-partition ops (illegal off 0/32/64/96 bases).

Engine facts this build enforces: tensor_tensor(min) and scans are
DVE-only (Pool rejects them); Pool runs tensor_scalar / copies / tt
add/mult; tensor_scalar is 4x on DVE (0.26 ns/elem bf16), tensor_tensor
2x (0.52). So both scans and all 16 D/H window mins run on DVE; Act and
Pool take the +d^2 bias copies, the BCE Ln, and finalize stages.

W pass: fwd scan over cols [0:28), bwd scan (reversed APs) over [4:32)
— each direction only needs its upstream 4-col halo; chained row
chunks; cross-row leaks land >= 5 = saturation.

BCE: host packs q = where(target, pred, 1-pred) (duplicated to both
partition parities) as fp16; device does scr = Ln(q + 3.8e-44) on Act
(the tiny bias reproduces torch's log clamp at -100 for q = 0);
bce = -scr, sign folded into the host reduction. The fp16 (not bf16)
ln/product chain keeps the systematic rounding bias ~1e-5.

Finalize per H-half: PE matmul with a banded pair-sum matrix combines
the interleaved channels (psum[2r] = g[2r] + g[2r+1] = a^2, exactly one
of the pair is 0); Act reads PSUM: da = Sqrt(0.25*a^2) = a/2; DVE:
w = clamp(2.5 - da, 0, 1) via 4x tensor_scalar, den/num accumulated
per-partition via tensor_scalar accum_out (accum reduces with op1, so
both accum ops end in add). Host sums EVEN partitions only (odd ones
carry parity-op garbage) in float64:
loss = mean_b(num_b / (den_b + 1e-5)).
"""

import numpy as np

B, D, H, W = 2, 64, 96, 96
NQ = 4
WI = W // NQ       # 24 interior columns per core
HALO = 4
WE = WI + 2 * HALO  # 32
N_CORES = B * NQ
HH = H // 2

_CACHE = {}


def _build():
    import concourse.bacc as bacc
    import concourse.mybir as mybir
    import concourse.tile as tile

    fp32 = mybir.dt.float32
    bf16 = mybir.dt.bfloat16
    fp16 = mybir.dt.float16
    AF = mybir.ActivationFunctionType
    ALU = mybir.AluOpType

    nc = bacc.Bacc("TRN2", target_bir_lowering=False, debug=False)
    t_d = nc.dram_tensor("t", [128, H, WE], bf16, kind="ExternalInput").ap()
    q_d = nc.dram_tensor("q", [128, H, WI], fp16, kind="ExternalInput").ap()
    s_d = nc.dram_tensor("s", [8, H, WI], bf16, kind="ExternalInput").ap()
    m_d = nc.dram_tensor("pm", [128, 128], bf16, kind="ExternalInput").ap()
    o_d = nc.dram_tensor("o", [128, 4], fp32, kind="ExternalOutput").ap()

    with tile.TileContext(nc) as tc:
        with (
            tc.tile_pool(name="mem", bufs=1) as pool,
            tc.tile_pool(name="ps", bufs=2, space="PSUM") as psp,
        ):
            t2 = pool.tile([128, H, WE], bf16)
            sf = pool.tile([128, H, WE], bf16)
            sb = pool.tile([128, H, WE], bf16)
            m1 = pool.tile([128, H, WI], bf16)
            sq = pool.tile([128, H, WI], bf16)
            thd = [pool.tile([128, H, WI], bf16, name=f"thd{d}") for d in (1, 2, 3, 4)]
            su = [pool.tile([128, H, WI], bf16, name=f"su{d}") for d in (1, 2, 3, 4)]
            sd = [pool.tile([128, H, WI], bf16, name=f"sd{d}") for d in (1, 2, 3, 4)]
            gd = pool.tile([128, H, WI], bf16)
            the = [pool.tile([128, H, WI], bf16, name=f"the{d}") for d in (1, 2, 3, 4)]
            gh = pool.tile([128, H, WI], bf16)
            qt = pool.tile([128, H, WI], fp16)
            scr = pool.tile([128, H, WI], fp16)
            da = pool.tile([128, H, WI], bf16)
            uv = pool.tile([128, H, WI], fp16)
            wv = pool.tile([128, H, WI], fp16)
            mb = pool.tile([128, H, WI], fp16)
            pm = pool.tile([128, 128], bf16)
            eps = pool.tile([128, 1], fp32)
            os_ = pool.tile([128, 4], fp32)

            def tmin(out_ap, a_ap, b_ap):
                nc.vector.tensor_tensor(out_ap, a_ap, b_ap, op=ALU.min)

            nc.vector.memset(eps[:], 3.7835058e-44)

            # ---- loads: upper rows first (bwd scan starts there), aux
            # tensors on the Act queue.
            nc.sync.dma_start(t2[:, 72:96], t_d[:, 72:96])
            nc.sync.dma_start(t2[:, 48:72], t_d[:, 48:72])
            nc.sync.dma_start(t2[:, 0:24], t_d[:, 0:24])
            nc.sync.dma_start(t2[:, 24:48], t_d[:, 24:48])
            nc.gpsimd.dma_start(qt[:], q_d)
            nc.gpsimd.dma_start(pm[:], m_d)
            for k, d in enumerate((1, 2, 3, 4)):
                nc.gpsimd.dma_start(su[k][128 - 2 * d:128], s_d[0:2 * d])
                nc.gpsimd.dma_start(sd[k][0:2 * d], s_d[0:2 * d])
            nc.scalar.activation(os_[:, 0:1], eps[:], AF.Ln)  # pin ln table

            # ---- W pass: fwd + bwd distance scans, both DVE, ordered to
            # chase the t2 chunk loads. state = t*state + t.
            def fscan(a, b, seed):
                src = t2[:, a:b].opt()
                nc.vector.tensor_tensor_scan(
                    sf[:, a:b].opt(), src, src, seed,
                    op0=ALU.mult, op1=ALU.add)

            def bscan(a, b, seed):
                src = t2[:, a:b].opt()[:, ::-1]
                nc.vector.tensor_tensor_scan(
                    sb[:, a:b].opt()[:, ::-1], src, src, seed,
                    op0=ALU.mult, op1=ALU.add)

            bscan(72, 96, 100.0)
            bscan(48, 72, sb[:, 72:73, 0:1].opt())
            fscan(0, 24, 100.0)
            fscan(24, 48, sf[:, 23:24, WE - 1:WE].opt())
            fscan(48, 96, sf[:, 47:48, WE - 1:WE].opt())
            # m1/sq upper half while the lower bwd chunks are still queued
            tmin(m1[:, HH:H], sf[:, HH:H, 4:28], sb[:, HH:H, 4:28])
            nc.vector.tensor_tensor(sq[:, HH:H], m1[:, HH:H], m1[:, HH:H],
                                    op=ALU.mult)
            nc.vector.tensor_scalar(thd[3][:, HH:H], sq[:, HH:H], 1.0, 16.0,
                                    op0=ALU.mult, op1=ALU.add)
            bscan(24, 48, sb[:, 48:49, 0:1].opt())
            bscan(0, 24, sb[:, 24:25, 0:1].opt())
            tmin(m1[:, 0:HH], sf[:, 0:HH, 4:28], sb[:, 0:HH, 4:28])
            nc.vector.tensor_tensor(sq[:, 0:HH], m1[:, 0:HH], m1[:, 0:HH],
                                    op=ALU.mult)
            nc.vector.tensor_scalar(thd[3][:, 0:HH], sq[:, 0:HH], 1.0, 16.0,
                                    op0=ALU.mult, op1=ALU.add)

            # ---- D pass: +d^2 biases (d=4 DVE halves above; d=3,2 Act;
            # d=1 DVE filler), per-half 2d-partition shift DMAs, 16
            # half-tile DVE mins ordered d=4..1, up/down, hi/lo.
            nc.scalar.activation(thd[2][:], sq[:], AF.Copy, bias=9.0)
            nc.scalar.activation(thd[1][:], sq[:], AF.Copy, bias=4.0)
            for d in (4, 3, 2):
                k = d - 1
                nc.sync.dma_start(su[k][0:128 - 2 * d], thd[k][2 * d:128])
                nc.sync.dma_start(sd[k][2 * d:128], thd[k][0:128 - 2 * d])
            tmin(gd[:], sq[:], su[3][:])
            # d=1 bias on DVE here: fills the gap while Act finishes thd3
            nc.vector.tensor_scalar(thd[0][:], sq[:], 1.0, 1.0,
                                    op0=ALU.mult, op1=ALU.add)
            nc.sync.dma_start(su[0][0:126], thd[0][2:128])
            nc.sync.dma_start(sd[0][2:128], thd[0][0:126])
            tmin(gd[:], sd[3][:], gd[:])
            for s_ in (su[2], sd[2], su[1], sd[1], su[0], sd[0]):
                tmin(gd[:], s_[:], gd[:])

            # ---- BCE on Act (q loaded long ago); pin the sqrt table
            # right after so the finalize Sqrt doesn't stall on a load.
            nc.scalar.activation(scr[:], qt[:], AF.Ln, bias=eps[:])
            nc.scalar.activation(os_[:, 1:2], eps[:], AF.Sqrt)

            # ---- H pass: free-dim shifted mins, d = 1..4, edge-trimmed.
            # Biases: d=1 DVE (needed first), d=2,3 Act, d=4 Pool.
            nc.vector.tensor_scalar(the[0][:], gd[:], 1.0, 1.0,
                                    op0=ALU.mult, op1=ALU.add)
            nc.scalar.activation(the[1][:], gd[:], AF.Copy, bias=4.0)
            nc.gpsimd.tensor_scalar(the[2][:], gd[:], 1.0, 9.0,
                                    op0=ALU.mult, op1=ALU.add)
            nc.scalar.activation(the[3][:], gd[:], AF.Copy, bias=16.0)
            # upper-half chain first so finalize(hi) overlaps the lower
            # chain; each half's ops read the[k] rows from both halves.
            tmin(gh[:, HH:96], gd[:, HH:96], the[0][:, HH - 1:95])
            tmin(gh[:, HH:95], the[0][:, HH + 1:96], gh[:, HH:95])
            tmin(gh[:, HH:96], the[1][:, HH - 2:94], gh[:, HH:96])
            tmin(gh[:, HH:94], the[1][:, HH + 2:96], gh[:, HH:94])
            tmin(gh[:, HH:96], the[2][:, HH - 3:93], gh[:, HH:96])
            tmin(gh[:, HH:93], the[2][:, HH + 3:96], gh[:, HH:93])
            tmin(gh[:, HH:96], the[3][:, HH - 4:92], gh[:, HH:96])
            tmin(gh[:, HH:92], the[3][:, HH + 4:96], gh[:, HH:92])
            nc.vector.tensor_copy(gh[:, 0:1, :], gd[:, 0:1, :])
            tmin(gh[:, 1:HH], gd[:, 1:HH], the[0][:, 0:HH - 1])
            tmin(gh[:, 0:HH], the[0][:, 1:HH + 1], gh[:, 0:HH])
            tmin(gh[:, 2:HH], the[1][:, 0:HH - 2], gh[:, 2:HH])
            tmin(gh[:, 0:HH], the[1][:, 2:HH + 2], gh[:, 0:HH])
            tmin(gh[:, 3:HH], the[2][:, 0:HH - 3], gh[:, 3:HH])
            tmin(gh[:, 0:HH], the[2][:, 3:HH + 3], gh[:, 0:HH])
            tmin(gh[:, 4:HH], the[3][:, 0:HH - 4], gh[:, 4:HH])
            tmin(gh[:, 0:HH], the[3][:, 4:HH + 4], gh[:, 0:HH])

            # ---- finalize per H-half (upper first): PE pair-sum, Act
            # sqrt from PSUM, DVE ramp + accumulates.
            FH = HH * WI  # 1152
            for i, (a, b) in enumerate(((HH, H), (0, HH))):
                pt = psp.tile([128, 3 * 512], fp32, name=f"ps{i}", tag="ps")
                ghf = gh[:, a:b, :].opt()
                for c0 in (0, 512, 1024):
                    c1 = min(c0 + 512, FH)
                    nc.tensor.matmul(pt[:, c0:c1], pm[:], ghf[:, c0:c1],
                                     start=True, stop=True)
                nc.scalar.activation(da[:, a:b, :].opt(), pt[:, 0:FH],
                                     AF.Sqrt, scale=0.25)
                nc.vector.tensor_scalar(uv[:, a:b], da[:, a:b], -1.0, 2.5,
                                        op0=ALU.mult, op1=ALU.add)
                nc.vector.tensor_scalar(wv[:, a:b], uv[:, a:b], 1.0, 0.0,
                                        op0=ALU.min, op1=ALU.max)
                nc.vector.tensor_scalar(uv[:, a:b], wv[:, a:b], 1.0, 0.0,
                                        op0=ALU.mult, op1=ALU.add,
                                        accum_out=os_[:, 2 * i:2 * i + 1])
                nc.vector.tensor_tensor(mb[:, a:b], wv[:, a:b], scr[:, a:b],
                                        op=ALU.mult)
                nc.vector.tensor_scalar(mb[:, a:b], mb[:, a:b], 1.0, 0.0,
                                        op0=ALU.mult, op1=ALU.add,
                                        accum_out=os_[:, 2 * i + 1:2 * i + 2])
                nc.sync.dma_start(o_d[:, 2 * i:2 * i + 2],
                                  os_[:, 2 * i:2 * i + 2])
    nc.compile()
    return nc


def _get_nc():
    if "nc" not in _CACHE:
        _CACHE["nc"] = _build()
    return _CACHE["nc"]


def _slabs(pred, target):
    import ml_dtypes

    bf16 = ml_dtypes.bfloat16
    tf = np.asarray(target, dtype=np.float32)
    tp = np.pad(tf, ((0, 0), (0, 0), (0, 0), (HALO, HALO)), mode="edge")
    qf = np.where(tf > 0.5, np.asarray(pred, dtype=np.float32),
                  1.0 - np.asarray(pred, dtype=np.float32))
    sent = np.full((8, H, WI), 100.0, dtype=bf16)
    pmat = np.zeros((128, 128), dtype=np.float32)
    for p in range(0, 128, 2):
        pmat[p, p] = 1.0
        pmat[p + 1, p] = 1.0
    pmat = pmat.astype(bf16)
    in_maps = []
    for b in range(B):
        for qr in range(NQ):
            ts_ = tp[b, :, :, qr * WI: qr * WI + WE]  # [64, H, WE]
            t2 = np.empty((128, H, WE), dtype=np.float32)
            t2[0::2] = ts_
            t2[1::2] = 1.0 - ts_
            qq = qf[b, :, :, qr * WI:(qr + 1) * WI]   # [64, H, WI]
            q2 = np.empty((128, H, WI), dtype=np.float32)
            q2[0::2] = qq
            q2[1::2] = qq
            in_maps.append({"t": t2.astype(bf16), "q": q2.astype(np.float16),
                            "s": sent, "pm": pmat})
    return in_maps


def kernel(pred: np.ndarray, target: np.ndarray) -> np.ndarray:
    from concourse.bass_utils import run_bass_kernel_spmd

    nc = _get_nc()
    in_maps = _slabs(pred, target)
    res = run_bass_kernel_spmd(nc, in_maps, list(range(N_CORES)))

    loss = 0.0
    for b in range(B):
        num = 0.0
        den = 0.0
        for qr in range(NQ):
            o = res.results[b * NQ + qr]["o"].astype(np.float64)[0::2]
            den += o[:, 0].sum() + o[:, 2].sum()
            num -= o[:, 1].sum() + o[:, 3].sum()
        loss += num / (den + 1e-5)
    return np.float32(loss / B)
